# revision 36
# baseline (speedup 1.0000x reference)
"""Longformer-style BERT (banded + global attention), 2 layers, on 8 TRN2
NeuronCores via Bass/Tile. Sequence-parallel: each core owns 512 tokens.

Per-core scheme (T=512 local tokens, E=1024 extended key window):
  - layer 0 is fully local: the host supplies embeddings for the extended
    window (own 512 + 256 halo each side, clamped) and for the 64 global
    rows, so no collective is needed before layer 1.
  - the only layer-boundary exchange is ONE 8-wide ReduceScatter delivering
    [256 left-halo + 256 right-halo + 64 global rows] of h^1 per core
    (each core scatters its boundary rows into its neighbours' slots and
    broadcasts its own global rows into every slot of a zero-filled
    staging buffer).
  - global-query attention is computed distributed (each core scores its
    own 512 keys) and combined with one small AllReduce per layer, issued
    early so it hides under the band attention.
  - every LN's scale/bias is folded into the NEXT matmul's weights on the
    host, so matmuls consume the pre-scale normalized value (zn) and the
    real h (residual path) is computed off the PE-gating chain.
  - the score-path projections (q, k, kg, qg) run as fp8e4m3 DoubleRow
    matmuls (weights host-scaled x128, undone in the psum drain); value
    paths stay bf16 since softmax damps absolute score noise but value
    noise passes straight through.
  - band attention: block-banded over 128-token tiles; query tile qt
    attends extended key tiles qt..qt+4. Triangle edge masks are additive
    -240 matrices accumulated into the score psums via PE matmuls
    (lhs=mask, rhs=identity); key validity is folded into v/vg/vgf as 0/1
    row masks, so exps are bias-free; per head-pair the [global | hi-edge]
    scores pack into one psum so 2 heads need 3 exps; a ones-column per
    head in v makes the PV matmul emit softmax denominators.
"""
import os
import sys

sys.path.insert(0, '/opt/trn_rl_repo')
sys.path.insert(0, os.path.dirname(os.path.abspath(__file__)))

import numpy as np
import ml_dtypes

import concourse.bass as bass
import concourse.tile as tile
from concourse import mybir
from concourse.bass_utils import run_bass_kernel_spmd

# ---- walrus sync-wait-limit workaround (inlined) ----
"""Workarounds for the pinned walrus build's per-instruction sync-wait limit.

This walrus errors with 'Too many sync wait commands' when an instruction
carries more than one sem wait. Two patches:

1. TileContext._lower_ordered_insts — before lowering, split any instruction
   with >MAXW on_wait entries: excess waits move to InstNoOp instructions
   inserted just before it on the same engine (engines are in-order, so
   waiting earlier on the same engine is always sound).

2. TileContext._drain_and_barrier — the end-of-kernel drain gets its waits
   spread over SP nops the same way.
"""
import concourse.tile as _tile
from concourse import mybir as _mybir
from concourse.vector_clock import ScopedClock as _ScopedClock

_MAXW = 1


def _split_waits_in_ordered(tc, ordered):
    nc = tc.nc
    for bb_name, insts in ordered.items():
        new_list = []
        for inst in insts:
            si = inst.sync_info
            waits = list(si.on_wait) if si is not None and si.on_wait else []
            if len(waits) > _MAXW and inst.engine != _mybir.EngineType.Unassigned:
                keep = waits[:_MAXW]
                extra = waits[_MAXW:]
                for j in range(0, len(extra), _MAXW):
                    nop = _mybir.InstNoOp(
                        name=nc.get_next_instruction_name(),
                        engine=inst.engine,
                        ins=[],
                        outs=[],
                        sync_info=_mybir.SyncInfo(
                            on_wait=extra[j:j + _MAXW], on_update=[]
                        ),
                        bass_nofuse=True,
                    )
                    nc.register_instruction(nop, overwrite=True)
                    new_list.append(nop)
                inst.sync_info = _mybir.SyncInfo(
                    on_wait=keep,
                    on_update=list(si.on_update) if si.on_update else [],
                )
            new_list.append(inst)
        ordered[bb_name] = new_list


_orig_lower = _tile.TileContext._lower_ordered_insts


def _patched_lower(self, ordered):
    _split_waits_in_ordered(self, ordered)
    return _orig_lower(self, ordered)


_tile.TileContext._lower_ordered_insts = _patched_lower


def _patched_drain_and_barrier(self, tick_clock, wait_clock):
    nc = self.nc
    drain_inst = nc.sync.drain()
    wait_clock.add_sem_waits(
        drain_inst.ins, _ScopedClock({None: tick_clock.global_clock})
    )
    si = drain_inst.ins.sync_info
    waits = list(si.on_wait) if si is not None and si.on_wait else []
    if len(waits) > _MAXW:
        drain_inst.ins.sync_info = _mybir.SyncInfo(
            on_wait=waits[:_MAXW],
            on_update=list(si.on_update) if si.on_update else [],
        )
        for i in range(_MAXW, len(waits), _MAXW):
            nop = nc.sync.nop(nofuse=True)
            nsi = nop.ins.sync_info
            nop.ins.sync_info = _mybir.SyncInfo(
                on_wait=waits[i:i + _MAXW],
                on_update=(list(nsi.on_update)
                           if (nsi is not None and nsi.on_update) else []),
            )
    nc.all_engine_barrier()
    assert self.sems is not None
    popped = nc._tile_sem_poison_stack.pop()
    assert popped is self._sem_poison
    nc.clear_and_free_semaphores(list(self.sems.allocated().values()))
    nc.all_engine_barrier()


_tile.TileContext._drain_and_barrier = _patched_drain_and_barrier


F32 = mybir.dt.float32
BF16 = mybir.dt.bfloat16
FP8 = mybir.dt.float8e4
DR = mybir.MatmulPerfMode.DoubleRow
WS = 128.0        # fp8 weight scale (undone in the psum drains)
WSI = 1.0 / WS
I32 = mybir.dt.int32
AF = mybir.ActivationFunctionType
AX = mybir.AxisListType
OP = mybir.AluOpType

NC_ = 8           # cores
S = 4096
D = 768
H = 12
FF = 3072
L = 2
T = S // NC_      # 512 tokens per core
QT = T // 128     # 4 query tiles per core
DT = D // 128     # 6 feature tiles
FT = FF // 128    # 24 ff tiles
ET = QT + 4       # 8 extended key tiles (halo 2 each side)
E = ET * 128      # 1024
NG = 64           # global tokens
RSROW = 2 * 256 + NG   # 576 rows per ReduceScatter slot
SCALE = 1.0 / 8.0
NEG = -30.0
TRI = -240.0      # pre-scale additive mask (TRI * SCALE = -30)
EPS = 1e-5

bfd = ml_dtypes.bfloat16
f8d = ml_dtypes.float8_e4m3


# ----------------------------------------------------------------------------
# device program
# ----------------------------------------------------------------------------

def build_program():
    nc = bass.Bass()

    def inp(name, shape, dtype=F32):
        return nc.declare_dram_parameter(name, list(shape), dtype,
                                         isOutput=False)

    t = {}
    t["e_sum"] = inp("e_sum", [E, D])           # ext-window embeddings
    t["e_g"] = inp("e_g", [NG, D])              # global-row embeddings
    for w in ("Wq", "Wk", "Wkg", "Wqg"):
        t[w] = inp(w, [L, D, D], FP8)
    for w in ("Wv", "Wvg", "Wo"):
        t[w] = inp(w, [L, D, D], BF16)
    t["Wf1"] = inp("Wf1", [L, D, FF], BF16)
    t["Wf2"] = inp("Wf2", [L, FF, D], BF16)
    for b in ("bq_p", "bk_p", "bkg_p", "bqg_p"):
        t[b] = inp(b, [L, 128, DT])
    t["bf1_p"] = inp("bf1_p", [L, 128, FT])
    for b in ("bv_b", "bvg_b", "bo_b", "bf2_b"):
        t[b] = inp(b, [L, 128, D], BF16)
    t["lnes_b"] = inp("lnes_b", [128, D], BF16)
    t["lneb_b"] = inp("lneb_b", [128, D], BF16)
    for b in ("ln1s_b", "ln1b_b", "ln2s_b", "ln2b_b"):
        t[b] = inp(b, [L, 128, D], BF16)
    t["kval01"] = inp("kval01", [128, ET])
    t["gkey01"] = inp("gkey01", [64, 1])
    t["fkey01"] = inp("fkey01", [128, QT])
    t["glb1m"] = inp("glb1m", [128, QT])
    t["sel"] = inp("sel", [QT, 64, 128], BF16)  # og scatter one-hots
    t["g_sel"] = inp("g_sel", [128, QT, NG], BF16)  # own-h -> hg gather
    t["bnd_idx"] = inp("bnd_idx", [128, QT], I32)   # rs_in scatter rows
    t["tri_lo"] = inp("tri_lo", [128, 128], BF16)   # -240 additive, lhs form
    t["tri_hi"] = inp("tri_hi", [128, 128], BF16)
    t["ident"] = inp("ident", [128, 128], BF16)
    t["out"] = nc.declare_dram_parameter("out", [T, D], F32, isOutput=True)

    with tile.TileContext(nc) as tc:
        with (
            tc.tile_pool(name="cn", bufs=1) as cn,
            tc.tile_pool(name="wp", bufs=1) as wp,
            tc.tile_pool(name="act", bufs=1) as act,
            tc.tile_pool(name="scr", bufs=1) as scr,
            tc.tile_pool(name="pTp", bufs=1) as pTp,
            tc.tile_pool(name="psp", bufs=1, space="PSUM") as psp,
            tc.tile_pool(name="dram", bufs=1, space="DRAM") as dram,
        ):
            _body(nc, t, cn, wp, act, scr, pTp, psp, dram)
    return nc


def _body(nc, t, cn, wp, act, scr, pTp, psp, dram):
    def load_const(name, shape, dtype=F32, eng=None):
        tl = cn.tile(list(shape), dtype, tag=name, name=name + "_sb")
        (eng or nc.scalar).dma_start(tl[:], t[name][:])
        return tl

    kval01 = load_const("kval01", [128, ET])
    lnes_b = load_const("lnes_b", [128, D], BF16)
    lneb_b = load_const("lneb_b", [128, D], BF16)
    gkey01 = load_const("gkey01", [64, 1], eng=nc.gpsimd)
    fkey01 = load_const("fkey01", [128, QT], eng=nc.gpsimd)
    glb1m = load_const("glb1m", [128, QT], eng=nc.gpsimd)
    ident = load_const("ident", [128, 128], BF16, eng=nc.gpsimd)
    tri_lo = load_const("tri_lo", [128, 128], BF16, eng=nc.gpsimd)
    tri_hi = load_const("tri_hi", [128, 128], BF16, eng=nc.gpsimd)
    g_sel = load_const("g_sel", [128, QT, NG], BF16, eng=nc.gpsimd)
    bnd_idx = load_const("bnd_idx", [128, QT], I32, eng=nc.gpsimd)
    sel_sb = cn.tile([64, QT, 128], BF16, tag="sel", name="sel_sb")
    nc.gpsimd.dma_start(sel_sb[:], t["sel"].rearrange("q g t -> g q t")[:])
    eps_c = cn.tile([128, 1], F32, tag="eps_c", name="eps_c")
    nc.vector.memset(eps_c[:], EPS)

    # ---- ReduceScatter staging: rs_in zero-filled once, single bcast DMA
    rs_in = dram.tile([NC_ * RSROW, D], BF16, tag="rs_in", name="rs_in")
    rs_out = dram.tile([RSROW, D], BF16, tag="rs_out", name="rs_out")
    zt = cn.tile([128, D], BF16, tag="zt", name="zt")
    nc.vector.memset(zt[:], 0.0)

    def emit_zero_fill():
        rz = rs_in.rearrange("(a p) d -> a p d", p=128)
        for zc in range(6):
            nc.gpsimd.dma_start(
                rz[zc * 6:(zc + 1) * 6, :, :].rearrange("a p d -> p a d")[:],
                zt[:].unsqueeze(1).broadcast_to([128, 6, D]))

    _cp = [0]

    def xcopy(dst, src_):
        _cp[0] += 1
        if _cp[0] % 2 == 0:
            nc.vector.tensor_copy(dst, src_)
        else:
            nc.scalar.copy(dst, src_)

    _pb = [0]

    def psum_bias_out(o_ap, ps_ap, b_col):
        # psum -> sbuf cast undoing the fp8 weight scale, plus bias
        _pb[0] += 1
        nc.vector.tensor_scalar(o_ap, ps_ap, WSI, b_col,
                                op0=OP.mult, op1=OP.add)

    def big32(name="b32"):
        return scr.tile([128, D], F32, tag="sD32", bufs=3, name=name)

    def small32(name="s32"):
        return scr.tile([128, 1], F32, tag="s1", bufs=6, name=name)

    def bigbf(name="bbf"):
        return scr.tile([128, D], BF16, tag="sDbf", bufs=1, name=name)

    # ---- layernorm: x fp32 [128, D]; zn_ap (bf16) gets the pre-scale
    # normalized value (consumed by matmuls whose weights have the LN
    # scale/bias host-folded in); real_ap (optional) gets zn*s+b for the
    # residual path, computed off the PE-gating chain.
    def layer_norm(x_tile, s_b, b_b, real_ap, zn_ap=None, rows=128):
        red = small32("ln_red")
        nc.vector.tensor_reduce(red[0:rows, :], x_tile[0:rows, :], axis=AX.X,
                                op=OP.add)
        mean = small32("ln_mean")
        nc.vector.tensor_scalar(mean[0:rows, :], red[0:rows, :], 1.0 / D,
                                None, op0=OP.mult)
        sq = scr.tile([128, D], F32, tag="ln_sq", bufs=1, name="ln_sq")
        ssq = small32("ln_ssq")
        nc.scalar.activation(sq[0:rows, :], x_tile[0:rows, :], AF.Square,
                             accum_out=ssq[0:rows, 0:1])
        # bias = eps - mean^2
        vb = small32("ln_vb")
        nc.vector.tensor_scalar(vb[0:rows, :], mean[0:rows, :],
                                mean[0:rows, 0:1], -1.0,
                                op0=OP.mult, op1=OP.mult)
        nc.vector.tensor_scalar_add(vb[0:rows, :], vb[0:rows, :],
                                    eps_c[0:rows, 0:1])
        std = small32("ln_std")
        nc.scalar.activation(std[0:rows, :], ssq[0:rows, :], AF.Sqrt,
                             bias=vb[0:rows, 0:1], scale=1.0 / D)
        rstd = small32("ln_rstd")
        nc.vector.reciprocal(rstd[0:rows, :], std[0:rows, :])
        if zn_ap is None:
            zn = big32("ln_zn")
            zn_ap = zn[0:rows, :]
        nc.vector.tensor_scalar(zn_ap, x_tile[0:rows, :],
                                mean[0:rows, 0:1], rstd[0:rows, 0:1],
                                op0=OP.subtract, op1=OP.mult)
        if real_ap is not None:
            tmp = big32("ln_tmp")
            nc.vector.tensor_mul(tmp[0:rows, :], zn_ap, s_b[0:rows, :])
            nc.vector.tensor_add(real_ap, tmp[0:rows, :], b_b[0:rows, :])

    # SBUF residency: own h (bf16) + LN1 output (f32) + halo/global bf16
    h_bf = [act.tile([128, D], BF16, tag=f"hbf_{i}", name=f"hbf_{i}")
            for i in range(QT)]
    h1 = [act.tile([128, D], F32, tag=f"h1_{i}", name=f"h1_{i}")
          for i in range(QT)]
    halo4 = act.tile([128, 4, D], BF16, tag="halo4", name="halo4")
    znb = [act.tile([128, D], BF16, tag=f"znb_{i}", name=f"znb_{i}")
           for i in range(QT)]
    hg_bf = act.tile([64, D], BF16, tag="hg_bf", name="hg_bf")

    # ---------------- layers ----------------
    for l in range(L):
        hT_ext = act.tile([128, DT, E], BF16, tag="hT_ext", name=f"hTe{l}")
        hT8 = act.tile([128, DT, E], FP8, tag="hT8", name=f"hT8{l}")

        def trans_into(src_ap, dst_cols, n=128):
            for d in range(DT):
                tps = psp.tile([128, 128], BF16, tag="p", bufs=8, name="tr")
                nc.tensor.transpose(tps[0:n, 0:n],
                                    src_ap[0:n, d * 128:(d + 1) * 128],
                                    ident[0:n, 0:n])
                nc.vector.tensor_copy(hT_ext[:, d, dst_cols:dst_cols + n],
                                      tps[:, 0:n])
                nc.scalar.copy(hT8[:, d, dst_cols:dst_cols + n], tps[:, 0:n])

        hgT = act.tile([128, DT, 64], BF16, tag="hgT", name=f"hgT{l}")
        hgT8 = act.tile([128, DT, 64], FP8, tag="hgT8", name=f"hgT8{l}")

        def trans_hg():
            for d in range(DT):
                tps = psp.tile([128, 128], BF16, tag="p", bufs=8, name="trg")
                nc.tensor.transpose(tps[:, 0:64],
                                    hg_bf[0:64, d * 128:(d + 1) * 128],
                                    ident[0:64, 0:64])
                nc.vector.tensor_copy(hgT[:, d, :], tps[:, 0:64])
                nc.scalar.copy(hgT8[:, d, :], tps[:, 0:64])

        # ---- l>0: halo + global rows arrive via the ReduceScatter result;
        # load them on Pool (queued right behind the RS itself)
        if l > 0:
            nc.gpsimd.dma_start(hg_bf[:], rs_out[512:512 + NG, :])
            for g in range(4):
                nc.gpsimd.dma_start(halo4[:, g, :],
                                    rs_out[g * 128:(g + 1) * 128, :])

        # ---- weight slabs (bulk on SP; FFN f2 on Pool; biases on Act)
        def wslab(src_ap, name, eng=None, dtype=BF16):
            tl = wp.tile([128, DT, D], dtype, tag="w", bufs=5, name=name)
            (eng or nc.sync).dma_start(
                tl[:], src_ap.rearrange("(k p) o -> p k o", p=128)[:])
            return tl

        slabs = {}

        def slab_feed(upto):
            # issue weight-slab DMAs lazily so the l=0 embedding loads
            # reach the DMA engines first
            order = [("wv", "Wv", BF16), ("wvg", "Wvg", BF16),
                     ("wq", "Wq", FP8), ("wk", "Wk", FP8),
                     ("wkg", "Wkg", FP8), ("wqg", "Wqg", FP8),
                     ("wo", "Wo", BF16)]
            for nm, key, dt_ in order[:upto]:
                if nm not in slabs:
                    slabs[nm] = wslab(t[key][l], f"{nm}{l}", dtype=dt_)

        if l > 0:
            slab_feed(7)
            w_v, w_vg, w_q, w_k, w_kg, w_qg, w_o = (
                slabs[n] for n in ("wv", "wvg", "wq", "wk", "wkg", "wqg",
                                   "wo"))

        def bload(name, n=DT, dtype=F32):
            tl = wp.tile([128, n], dtype, tag=f"b_{name}", name=f"{name}{l}")
            nc.scalar.dma_start(tl[:], t[name][l][:])
            return tl

        b_v = bload("bv_b", D, BF16)
        b_vg = bload("bvg_b", D, BF16)
        b_q = bload("bq_p")
        b_kg = bload("bkg_p")
        b_k = bload("bk_p")
        lateb = {}

        def late_bloads():
            lateb["b_qg"] = bload("bqg_p")
            lateb["b_f1"] = bload("bf1_p", FT)
            lateb["b_o"] = bload("bo_b", D, BF16)
            lateb["b_f2"] = bload("bf2_b", D, BF16)
            lateb["ln1s"] = bload("ln1s_b", D, BF16)
            lateb["ln1b"] = bload("ln1b_b", D, BF16)
            lateb["ln2s"] = bload("ln2s_b", D, BF16)
            lateb["ln2b"] = bload("ln2b_b", D, BF16)

        if l > 0:
            late_bloads()

        # token-major projection into head-payload layout (65-stride) with
        # a ones column; bias folded in a single strided tensor_add
        def proj_tm(w_sb, b_bc, o, tt, tcol):
            for c0 in (0, 512):
                c1 = min(c0 + 512, D)
                ps = psp.tile([128, 512], F32, tag="p", bufs=8, name="pt")
                for k in range(DT):
                    nc.tensor.matmul(
                        ps[:, 0:c1 - c0],
                        hT_ext[:, k, tcol:tcol + 128],
                        w_sb[:, k, c0:c1],
                        start=(k == 0), stop=(k == DT - 1))
                nh = (c1 - c0) // 64
                h0 = c0 // 64
                dst = o[:, tt].rearrange("p (hh c) -> p hh c", c=65)
                nc.vector.tensor_add(
                    dst[:, h0:h0 + nh, 0:64],
                    ps[:, 0:c1 - c0].rearrange("p (hh c) -> p hh c", c=64)[:],
                    b_bc[:, c0:c1].rearrange("p (hh c) -> p hh c", c=64)[:])
            nc.gpsimd.memset(
                o[:, tt].rearrange("p (hh c) -> p hh c", c=65)
                [:, :, 64:65], 1.0)

        v_sb = act.tile([128, ET, H * 65], BF16, tag="v_sb", name=f"v{l}")
        vgf_sb = act.tile([128, QT, H * 65], BF16, tag="vgf_sb",
                          name=f"vgf{l}")

        def v_tile(tt):
            # band-v for ext tile tt, with key-validity folded in
            proj_tm(slabs["wv"], b_v, v_sb, tt, tt * 128)
            nc.vector.tensor_scalar_mul(v_sb[:, tt, :], v_sb[:, tt, :],
                                        kval01[:, tt:tt + 1])

        def vgf_tile(qt):
            proj_tm(slabs["wvg"], b_vg, vgf_sb, qt, 256 + qt * 128)
            nc.vector.tensor_scalar_mul(vgf_sb[:, qt, :], vgf_sb[:, qt, :],
                                        fkey01[:, qt:qt + 1])

        # ---- feature-major projections over own tokens
        def proj_fm(w_sb, b_sb, o, rc0, w, oc0):
            for cc in range(0, w, 512):
                cw = min(cc + 512, w) - cc
                for ot in range(DT):
                    ps = psp.tile([128, 512], F32, tag="p", bufs=8,
                                  name="pw")
                    for k in range(0, DT, 2):
                        nc.tensor.matmul(
                            ps[:, 0:cw],
                            w_sb[:, k:k + 2, ot * 128:(ot + 1) * 128],
                            hT8[:, k:k + 2, rc0 + cc:rc0 + cc + cw],
                            start=(k == 0), stop=(k == DT - 2),
                            perf_mode=DR)
                    psum_bias_out(o[:, ot, oc0 + cc:oc0 + cc + cw],
                                  ps[:, 0:cw], b_sb[:, ot:ot + 1])

        if l == 0:
            qT_e = act.tile([128, DT, T], BF16, tag="qT", name=f"qT{l}")
            kgfT_e = act.tile([128, DT, T], BF16, tag="kgfT",
                              name=f"kgfT{l}")

        kT = act.tile([128, DT, E], BF16, tag="kT", name=f"kT{l}")

        def own_proj_early():
            proj_fm(slabs["wq"], b_q, qT_e, 256, T, 0)
            proj_fm(slabs["wkg"], b_kg, kgfT_e, 256, T, 0)
            proj_fm(slabs["wk"], b_k, kT, 256, T, 256)

        # ---- embedding (l=0) or own transposes (l=1), pipelined with the
        # per-tile v projections so PE fills early
        if l == 0:
            for qt in range(QT):
                es = big32("emb_e")
                nc.sync.dma_start(es[:],
                                  t["e_sum"][(qt + 2) * 128:(qt + 3) * 128, :])
                slab_feed(qt + 2)
                layer_norm(es, lnes_b, lneb_b, h_bf[qt][:], znb[qt][:])
                trans_into(znb[qt][:], 256 + qt * 128)
                v_tile(qt + 2)
                vgf_tile(qt)
            eg = big32("emb_g")
            nc.scalar.dma_start(eg[0:64, :], t["e_g"][:])
            slab_feed(5)
            layer_norm(eg, None, None, None, hg_bf[0:64, :], rows=64)
            trans_hg()
            own_proj_early()
            for g in range(4):
                ecol = g * 128 if g < 2 else (g + 4) * 128
                es = big32("emb_h")
                nc.sync.dma_start(es[:], t["e_sum"][ecol:ecol + 128, :])
                slab_feed(6 + g)
                layer_norm(es, None, None, None, halo4[:, g, :])
                trans_into(halo4[:, g, :], ecol)
                v_tile(g if g < 2 else g + 4)
            slab_feed(7)
            w_v, w_vg, w_q, w_k, w_kg, w_qg, w_o = (
                slabs[n] for n in ("wv", "wvg", "wq", "wk", "wkg", "wqg",
                                   "wo"))
        else:
            for qt in range(QT):
                trans_into(znb[qt][:], 256 + qt * 128)
                v_tile(qt + 2)
                vgf_tile(qt)

        if l == 0:
            late_bloads()
        b_qg = lateb["b_qg"]
        b_f1 = lateb["b_f1"]
        b_o = lateb["b_o"]
        b_f2 = lateb["b_f2"]
        ln1s = lateb["ln1s"]
        ln1b = lateb["ln1b"]
        ln2s = lateb["ln2s"]
        ln2b = lateb["ln2b"]
        if l == 0:
            qT = qT_e
            kgfT = kgfT_e
        else:
            qT = act.tile([128, DT, T], BF16, tag="qT", name=f"qT{l}")
            proj_fm(w_q, b_q, qT, 256, T, 0)
            kgfT = act.tile([128, DT, T], BF16, tag="kgfT", name=f"kgfT{l}")
            proj_fm(w_kg, b_kg, kgfT, 256, T, 0)
        if l > 0:
            proj_fm(w_k, b_k, kT, 256, T, 256)
            # halo-dependent parts, queued behind the ReduceScatter
            for g in range(4):
                ecol = g * 128 if g < 2 else (g + 4) * 128
                trans_into(halo4[:, g, :], ecol)
                v_tile(g if g < 2 else g + 4)
            trans_hg()

        def proj_fm_g(w_sb, b_sb, tag):
            o = act.tile([128, DT, 64], BF16, tag=tag, name=tag + str(l))
            for ot in range(DT):
                ps = psp.tile([128, 512], F32, tag="p", bufs=8, name="pg_")
                for k in range(0, DT, 2):
                    nc.tensor.matmul(
                        ps[:, 0:64],
                        w_sb[:, k:k + 2, ot * 128:(ot + 1) * 128],
                        hgT8[:, k:k + 2, :],
                        start=(k == 0), stop=(k == DT - 2),
                        perf_mode=DR)
                psum_bias_out(o[:, ot, :], ps[:, 0:64], b_sb[:, ot:ot + 1])
            return o

        qgT = proj_fm_g(w_qg, b_qg, "qgT")
        kgT = proj_fm_g(w_k, b_k, "kgT")

        vg_sb = act.tile([64, H * 65], BF16, tag="vg_sb", name=f"vg{l}")
        for c0 in (0, 512):
            c1 = min(c0 + 512, D)
            ps = psp.tile([128, 512], F32, tag="p", bufs=8, name="pvg")
            for k in range(DT):
                nc.tensor.matmul(ps[0:64, 0:c1 - c0], hgT[:, k, :],
                                 w_v[:, k, c0:c1],
                                 start=(k == 0), stop=(k == DT - 1))
            nh = (c1 - c0) // 64
            h0 = c0 // 64
            dst = vg_sb.rearrange("p (hh c) -> p hh c", c=65)
            nc.vector.tensor_add(
                dst[:, h0:h0 + nh, 0:64],
                ps[0:64, 0:c1 - c0].rearrange("p (hh c) -> p hh c", c=64)[:],
                b_v[0:64, c0:c1].rearrange("p (hh c) -> p hh c", c=64)[:])
        nc.gpsimd.memset(
            vg_sb.rearrange("p (hh c) -> p hh c", c=65)[:, :, 64:65], 1.0)
        nc.vector.tensor_scalar_mul(vg_sb[:], vg_sb[:], gkey01[:, 0:1])

        # ---- global-query attention partials + AllReduce (issued early so
        # the collective overlaps the band attention below)
        stag = scr.tile([64, H, 65], BF16, tag="gq_stage", name=f"stag{l}")
        for hh in range(H):
            hp, hr = hh // 2, (hh % 2) * 64
            prow = slice(hr, hr + 64)
            spf = psp.tile([128, 256], F32, tag="p", bufs=8, name="spf")
            for kt in range(QT):
                nc.tensor.matmul(
                    spf[:, kt * 64:(kt + 1) * 64],
                    kgfT[prow, hp, kt * 128:(kt + 1) * 128],
                    qgT[prow, hp, :], start=True, stop=True)
            pfa = pTp.tile([128, 256], BF16, tag="pfa", bufs=3, name="pfa")
            nc.scalar.activation(pfa[:], spf[:], AF.Exp, scale=SCALE)
            gps = psp.tile([128, 65], F32, tag="p", bufs=8, name="gps")
            for kt in range(QT):
                nc.tensor.matmul(gps[0:64, :],
                                 pfa[:, kt * 64:(kt + 1) * 64],
                                 vgf_sb[:, kt, hh * 65:(hh + 1) * 65],
                                 start=(kt == 0), stop=(kt == QT - 1))
            nc.vector.tensor_copy(stag[:, hh, :], gps[0:64, :])
        cc2_in = dram.tile([64, H * 65], BF16, tag="cc2_in", name=f"c2i{l}")
        cc2_out = dram.tile([64, H * 65], BF16, tag="cc2_out",
                             name=f"c2o{l}", addr_space="Shared")
        nc.sync.dma_start(cc2_in[:], stag.rearrange("p a b -> p (a b)")[:])
        nc.gpsimd.collective_compute(
            "AllReduce", OP.add,
            ins=[cc2_in[:]], outs=[cc2_out[:]],
            replica_groups=[list(range(NC_))],
        )
        if l == 0:
            emit_zero_fill()
        gsum = scr.tile([64, H, 65], BF16, tag="gq_sum", name=f"gsum{l}")
        nc.sync.dma_start(gsum.rearrange("p a b -> p (a b)")[:], cc2_out[:])

        proj_fm(w_k, b_k, kT, 0, 256, 0)
        proj_fm(w_k, b_k, kT, 768, 256, 768)

        # ---- band + global-key attention -> o_sb
        o_sb = act.tile([128, QT, D], BF16, tag="o_sb", name=f"osb{l}")
        og = act.tile([64, D], BF16, tag="og", name=f"og{l}")
        h1T = act.tile([128, DT, T], BF16, tag="kgfT", name=f"h1T{l}")
        for qt in range(QT):
            qsl = slice(qt * 128, (qt + 1) * 128)
            for hp in range(H // 2):
                # head pair (2*hp, 2*hp+1): one packed psum holds both
                # heads' [global-key | hi-edge] scores -> one exp covers
                # both; each head keeps its own 4-tile band psum.
                sgh = psp.tile([128, 512], F32, tag="p", bufs=8, name="sgh")
                sp4s = []
                for sub in range(2):
                    hh = 2 * hp + sub
                    prow = slice(sub * 64, sub * 64 + 64)
                    c0 = sub * 256
                    nc.tensor.matmul(sgh[0:64, c0:c0 + 128],
                                     kgT[prow, hp, :],
                                     qT[prow, hp, qsl], start=True, stop=True)
                    nc.tensor.matmul(
                        sgh[:, c0 + 128:c0 + 256],
                        kT[prow, hp, (qt + 4) * 128:(qt + 5) * 128],
                        qT[prow, hp, qsl], start=True, stop=False)
                    nc.tensor.matmul(sgh[:, c0 + 128:c0 + 256],
                                     tri_hi[:], ident[:],
                                     start=False, stop=True)
                    sp4 = psp.tile([128, 512], F32, tag="p", bufs=8,
                                   name="sp4")
                    nc.tensor.matmul(
                        sp4[:, 0:128], kT[prow, hp, qt * 128:(qt + 1) * 128],
                        qT[prow, hp, qsl], start=True, stop=False)
                    nc.tensor.matmul(sp4[:, 0:128], tri_lo[:], ident[:],
                                     start=False, stop=True)
                    for a in range(1, 4):
                        e = qt + a
                        nc.tensor.matmul(
                            sp4[:, a * 128:(a + 1) * 128],
                            kT[prow, hp, e * 128:(e + 1) * 128],
                            qT[prow, hp, qsl], start=True, stop=True)
                    sp4s.append(sp4)
                ptg = pTp.tile([128, 512], BF16, tag="ptg", bufs=3,
                               name="ptg")
                nc.scalar.activation(ptg[:], sgh[:], AF.Exp, scale=SCALE)
                pt4s = []
                for sub in range(2):
                    pt4 = pTp.tile([128, 512], BF16, tag="pt4", bufs=6,
                                   name="pt4")
                    nc.scalar.activation(pt4[:], sp4s[sub][:], AF.Exp,
                                         scale=SCALE)
                    pt4s.append(pt4)
                for sub in range(2):
                    hh = 2 * hp + sub
                    c0 = sub * 256
                    ops = psp.tile([128, 65], F32, tag="p", bufs=8,
                                   name="ops")
                    nc.tensor.matmul(ops[:], ptg[0:64, c0:c0 + 128],
                                     vg_sb[:, hh * 65:(hh + 1) * 65],
                                     start=True, stop=False)
                    for a in range(4):
                        e = qt + a
                        nc.tensor.matmul(
                            ops[:], pt4s[sub][:, a * 128:(a + 1) * 128],
                            v_sb[:, e, hh * 65:(hh + 1) * 65],
                            start=False, stop=False)
                    nc.tensor.matmul(
                        ops[:], ptg[:, c0 + 128:c0 + 256],
                        v_sb[:, qt + 4, hh * 65:(hh + 1) * 65],
                        start=False, stop=True)
                    rec = small32("rec")
                    nc.vector.reciprocal(rec[:], ops[:, 64:65])
                    nc.vector.tensor_scalar(
                        o_sb[:, qt, hh * 64:(hh + 1) * 64], ops[:, 0:64],
                        rec[:, 0:1], glb1m[:, qt:qt + 1],
                        op0=OP.mult, op1=OP.mult)

        # ---- og divisions + scatter, then per-qt [oT-trans, Wo] with the
        # h1T transposes staggered one tile behind their LN1
        rec12 = scr.tile([64, H], F32, tag="rec12", name=f"rec12{l}")
        nc.vector.reciprocal(rec12[:], gsum[:, :, 64])
        nc.vector.tensor_mul(
            og.rearrange("p (hh c) -> p hh c", c=64)[:],
            gsum[:, :, 0:64],
            rec12[:].unsqueeze(2).broadcast_to([64, H, 64]))
        for qt in range(QT):
            for c0 in (0, 512):
                c1 = min(c0 + 512, D)
                sc = psp.tile([128, 512], F32, tag="p", bufs=8, name="sc")
                nc.tensor.matmul(sc[:, 0:c1 - c0], sel_sb[:, qt, :],
                                 og[:, c0:c1], start=True, stop=True)
                nc.vector.tensor_add(o_sb[:, qt, c0:c1], o_sb[:, qt, c0:c1],
                                     sc[:, 0:c1 - c0])
        zn1s = []
        for qt in range(QT):
            oTq = scr.tile([128, DT, 128], BF16, tag="oTq", bufs=2,
                           name=f"oTq{qt}")
            for d in range(DT):
                tps = psp.tile([128, 128], BF16, tag="p", bufs=8, name="tro")
                nc.tensor.transpose(
                    tps[:], o_sb[:, qt, d * 128:(d + 1) * 128], ident[:])
                xcopy(oTq[:, d, :], tps[:])
            x1 = big32("x1")
            nc.vector.tensor_add(x1[:], h_bf[qt][:], b_o[:])
            for c0 in (0, 512):
                c1 = min(c0 + 512, D)
                ps = psp.tile([128, 512], F32, tag="p", bufs=8, name="pwo")
                for k in range(DT):
                    nc.tensor.matmul(
                        ps[:, 0:c1 - c0], oTq[:, k, :],
                        w_o[:, k, c0:c1],
                        start=(k == 0), stop=(k == DT - 1))
                nc.vector.tensor_add(x1[:, c0:c1], x1[:, c0:c1],
                                     ps[:, 0:c1 - c0])
            zn1 = scr.tile([128, D], BF16, tag="zn1", bufs=2,
                           name=f"zn1_{qt}")
            layer_norm(x1, ln1s, ln1b, h1[qt][:], zn1[:])
            zn1s.append(zn1)
            if qt >= 1:
                for d in range(DT):
                    tps = psp.tile([128, 128], BF16, tag="p", bufs=8,
                                   name="trh1")
                    nc.tensor.transpose(
                        tps[:], zn1s[qt - 1][:, d * 128:(d + 1) * 128],
                        ident[:])
                    xcopy(h1T[:, d, (qt - 1) * 128:qt * 128], tps[:])
        for d in range(DT):
            tps = psp.tile([128, 128], BF16, tag="p", bufs=8, name="trh1")
            nc.tensor.transpose(
                tps[:], zn1s[QT - 1][:, d * 128:(d + 1) * 128], ident[:])
            xcopy(h1T[:, d, (QT - 1) * 128:QT * 128], tps[:])

        # ---- FFN: x2 accumulates in-place on h1 (f32); LN2 + output
        # staging fold into the second half's f2 loop so the boundary
        # exchange can start as early as possible
        for qt in range(QT):
            nc.vector.tensor_add(h1[qt][:], h1[qt][:], b_f2[:])
        for half in range(2):
            f1a = wslab(t["Wf1"][l][:, half * 1536:half * 1536 + 768],
                        f"f1a{l}{half}")
            f1b = wslab(t["Wf1"][l][:, half * 1536 + 768:(half + 1) * 1536],
                        f"f1b{l}{half}")
            f2a = wslab(t["Wf2"][l][half * 1536:half * 1536 + 768, :],
                        f"f2a{l}{half}", eng=nc.gpsimd)
            f2b = wslab(t["Wf2"][l][half * 1536 + 768:(half + 1) * 1536, :],
                        f"f2b{l}{half}", eng=nc.gpsimd)
            gT = act.tile([128, FT // 2, T], BF16, tag="v_sb", bufs=1,
                          name=f"gT{l}{half}")
            for ft in range(FT // 2):
                fabs = half * (FT // 2) + ft
                slab = f1a if ft < 6 else f1b
                ps = psp.tile([128, 512], F32, tag="p", bufs=8, name="pf1")
                for tc in (0, 256):
                    for k in range(DT):
                        nc.tensor.matmul(
                            ps[:, tc:tc + 256],
                            slab[:, k, (ft % 6) * 128:(ft % 6 + 1) * 128],
                            h1T[:, k, tc:tc + 256],
                            start=(k == 0), stop=(k == DT - 1))
                nc.scalar.activation(gT[:, ft, :], ps[:], AF.Gelu_apprx_tanh,
                                     bias=b_f1[:, fabs:fabs + 1])
            for qt in range(QT):
                for c0 in (0, 512):
                    c1 = min(c0 + 512, D)
                    ps = psp.tile([128, 512], F32, tag="p", bufs=8,
                                  name="pf2")
                    for k in range(FT // 2):
                        slab = f2a if k < 6 else f2b
                        nc.tensor.matmul(
                            ps[:, 0:c1 - c0],
                            gT[:, k, qt * 128:(qt + 1) * 128],
                            slab[:, k % 6, c0:c1],
                            start=(k == 0), stop=(k == FT // 2 - 1))
                    nc.vector.tensor_add(h1[qt][:, c0:c1], h1[qt][:, c0:c1],
                                         ps[:, 0:c1 - c0])
                if half == 1:
                    if l + 1 < L:
                        layer_norm(h1[qt], ln2s, ln2b, h_bf[qt][:],
                                   znb[qt][:])
                        nc.gpsimd.indirect_dma_start(
                            out=rs_in[:],
                            out_offset=bass.IndirectOffsetOnAxis(
                                ap=bnd_idx[:, qt:qt + 1], axis=0),
                            in_=znb[qt][:], in_offset=None,
                        )
                    else:
                        hout = big32("hout")
                        layer_norm(h1[qt], ln2s, ln2b, hout[:])
                        nc.sync.dma_start(
                            t["out"][qt * 128:(qt + 1) * 128, :], hout[:])

        if l + 1 < L:
            # own global rows: hg_own = sum_qt g_sel[:,qt,:]^T @ h_bf[qt],
            # broadcast into every slot's global section, then ReduceScatter
            hg_stage = bigbf("hg_stage")
            for c0 in (0, 512):
                c1 = min(c0 + 512, D)
                ps = psp.tile([128, 512], F32, tag="p", bufs=8, name="phg")
                for qt in range(QT):
                    nc.tensor.matmul(ps[0:64, 0:c1 - c0], g_sel[:, qt, :],
                                     znb[qt][:, c0:c1],
                                     start=(qt == 0), stop=(qt == QT - 1))
                nc.vector.tensor_copy(hg_stage[0:64, c0:c1],
                                      ps[0:64, 0:c1 - c0])
            nc.sync.dma_start(
                rs_in.rearrange("(s r) d -> r s d", r=RSROW)
                [512:512 + NG, :, :],
                hg_stage[0:64, :].unsqueeze(1).broadcast_to([64, NC_, D]))
            nc.gpsimd.collective_compute(
                "ReduceScatter", OP.add,
                ins=[rs_in[:]], outs=[rs_out[:]],
                replica_groups=[list(range(NC_))],
            )


# ----------------------------------------------------------------------------
# host side
# ----------------------------------------------------------------------------

_prog_cache = {}


def _get_program():
    if "nc" not in _prog_cache:
        _prog_cache["nc"] = build_program()
    return _prog_cache["nc"]


def _prep_maps(inputs):
    gi = {k: np.asarray(v) for k, v in inputs.items()}
    x = gi["x"][0]
    segs = gi["segs"][0]
    mask = gi["mask_src"][0] > 0
    clss = gi["clss"][0]

    is_glb = np.zeros(S, bool)
    is_glb[clss] = True

    def bcast(v, dt=np.float32):
        v = np.asarray(v, np.float32)
        return np.broadcast_to(v[None, :], (128, v.shape[0])).astype(dt)

    def part(v):
        return np.asarray(v, np.float32).reshape(-1, 128).T.copy()

    ar = np.arange(128)
    # fold the preceding LN's scale/bias into each projection so the device
    # can feed pre-scale zn activations straight into the matmuls:
    #   W' = diag(s_prev) @ W ; b' = b_prev @ W + b
    prev_s = [np.asarray(gi["ln_e_s"], np.float64),
              np.asarray(gi["ln2_s"][0], np.float64)]
    prev_b = [np.asarray(gi["ln_e_b"], np.float64),
              np.asarray(gi["ln2_b"][0], np.float64)]

    def fold(Wk_, bk_, s_l, b_l):
        W = np.asarray(gi[Wk_], np.float64)
        b = np.asarray(gi[bk_], np.float64)
        Wf = np.stack([s_l[l][:, None] * W[l] for l in range(L)])
        bf = np.stack([b_l[l] @ W[l] + b[l] for l in range(L)])
        return Wf.astype(np.float32), bf.astype(np.float32)

    Wq_f, bq_f = fold("Wq", "bq", prev_s, prev_b)
    Wk_f, bk_f = fold("Wk", "bk", prev_s, prev_b)
    Wv_f, bv_f = fold("Wv", "bv", prev_s, prev_b)
    Wqg_f, bqg_f = fold("Wqg", "bqg", prev_s, prev_b)
    Wkg_f, bkg_f = fold("Wkg", "bkg", prev_s, prev_b)
    Wvg_f, bvg_f = fold("Wvg", "bvg", prev_s, prev_b)
    ln1_s = [np.asarray(gi["ln1_s"][l], np.float64) for l in range(L)]
    ln1_b = [np.asarray(gi["ln1_b"][l], np.float64) for l in range(L)]
    Wf1_f, bf1_f = fold("Wf1", "bf1", ln1_s, ln1_b)
    shared = {
        "Wq": (Wq_f * WS).astype(f8d), "Wk": (Wk_f * WS).astype(f8d),
        "Wv": Wv_f.astype(bfd), "Wqg": (Wqg_f * WS).astype(f8d),
        "Wkg": (Wkg_f * WS).astype(f8d), "Wvg": Wvg_f.astype(bfd),
        "Wo": gi["Wo"].astype(bfd),
        "Wf1": Wf1_f.astype(bfd), "Wf2": gi["Wf2"].astype(bfd),
        "bq_p": np.stack([part(bq_f[l]) for l in range(L)]),
        "bk_p": np.stack([part(bk_f[l]) for l in range(L)]),
        "bkg_p": np.stack([part(bkg_f[l]) for l in range(L)]),
        "bqg_p": np.stack([part(bqg_f[l]) for l in range(L)]),
        "bf1_p": np.stack([part(bf1_f[l]) for l in range(L)]),
        "bv_b": np.stack([bcast(bv_f[l], bfd) for l in range(L)]),
        "bvg_b": np.stack([bcast(bvg_f[l], bfd) for l in range(L)]),
        "bo_b": np.stack([bcast(gi["bo"][l], bfd) for l in range(L)]),
        "bf2_b": np.stack([bcast(gi["bf2"][l], bfd) for l in range(L)]),
        "lnes_b": bcast(gi["ln_e_s"], bfd), "lneb_b": bcast(gi["ln_e_b"], bfd),
        "ln1s_b": np.stack([bcast(gi["ln1_s"][l], bfd) for l in range(L)]),
        "ln1b_b": np.stack([bcast(gi["ln1_b"][l], bfd) for l in range(L)]),
        "ln2s_b": np.stack([bcast(gi["ln2_s"][l], bfd) for l in range(L)]),
        "ln2b_b": np.stack([bcast(gi["ln2_b"][l], bfd) for l in range(L)]),
        "gkey01": mask[clss].astype(np.float32).reshape(64, 1),
        # additive -240 triangle masks in lhs-matmul form: psum[r, q] +=
        # lhs[q, r] (rhs = identity), so index [partition=q, free=r]
        "tri_lo": np.where(ar[None, :] < ar[:, None], TRI, 0.0).astype(bfd),
        "tri_hi": np.where(ar[None, :] > ar[:, None], TRI, 0.0).astype(bfd),
        "ident": np.eye(128, dtype=bfd),
        "e_g": (gi["word_emb"][x[clss]] + gi["pos_emb"][clss]
                + gi["type_emb"][segs[clss]]).astype(np.float32),
    }

    # scatter representative: one entry per position (duplicates collapse)
    rep = np.zeros(64, bool)
    seen = set()
    for g in range(63, -1, -1):
        if int(clss[g]) not in seen:
            seen.add(int(clss[g]))
            rep[g] = True

    maps = []
    for c in range(NC_):
        s0, s1 = c * T, (c + 1) * T
        toks = np.arange(s0, s1)
        ext = np.arange(s0 - 256, s1 + 256)
        ext_ok = (ext >= 0) & (ext < S)
        extc = np.clip(ext, 0, S - 1)
        kval = np.where(ext_ok & mask[extc] & ~is_glb[extc], 0.0, NEG)
        sel = np.zeros((QT, 64, 128), np.float32)
        for g in range(64):
            p = int(clss[g])
            if rep[g] and s0 <= p < s1:
                sel[(p - s0) // 128, g, (p - s0) % 128] = 1.0
        # own-h -> hg gather one-hots (all 64 columns, duplicates included)
        g_sel = np.zeros((QT, 128, 64), np.float32)
        for g in range(64):
            p = int(clss[g])
            if s0 <= p < s1:
                g_sel[(p - s0) // 128, (p - s0) % 128, g] = 1.0
        # rs_in destination rows: own first 256 -> left neighbour's slot
        # right-halo section; own last 256 -> right neighbour's slot
        # left-halo section
        bnd = np.zeros((QT, 128), np.int64)
        for qt in range(QT):
            rows = np.arange(qt * 128, (qt + 1) * 128)
            if qt < 2:
                bnd[qt] = ((c - 1) % NC_) * RSROW + 256 + rows
            else:
                bnd[qt] = ((c + 1) % NC_) * RSROW + (rows - 256)
        m = {
            "e_sum": (gi["word_emb"][x[extc]] + gi["pos_emb"][extc]
                      + gi["type_emb"][segs[extc]]).astype(np.float32),
            "kval01": (kval == 0.0).astype(np.float32)
                        .reshape(ET, 128).T.copy(),
            "fkey01": mask[toks].astype(np.float32)
                        .reshape(QT, 128).T.copy(),
            "glb1m": (~is_glb[toks]).astype(np.float32)
                       .reshape(QT, 128).T.copy(),
            "sel": sel.astype(bfd),
            "g_sel": g_sel.transpose(1, 0, 2).astype(bfd).copy(),
            "bnd_idx": bnd.T.astype(np.int32).copy(),
        }
        m.update(shared)
        maps.append(m)
    return maps


def kernel(**inputs):
    nc = _get_program()
    maps = _prep_maps(inputs)
    res = run_bass_kernel_spmd(nc, maps, list(range(NC_)))
    out = np.concatenate([res.results[c]["out"] for c in range(NC_)], axis=0)
    return out[None].astype(np.float32)
